# revision 1
# baseline (speedup 1.0000x reference)
"""Bilateral filter (K=7, guide channels=3) Trainium2 Bass kernel.

Contract: kernel(**inputs) takes FULL unsharded numpy inputs
(input [2,32,256,256] f32, input_for_kernel [2,3,256,256] f32,
sigma_for_kernel scalar f32) and returns the full output [2,32,256,256] f32.
Shards internally over 8 NeuronCores: (batch=2) x (4 h-blocks of 64 rows).

Math (identical to the reference up to fp rounding; the spatial-gaussian
normalization cancels in ker/norm):
  m_u[p]   = exp(-0.5*sum_c (g[c,p+u]-g[c,p])^2) * exp(-0.5*(uy^2+ux^2)/s^2)
  out[c,p] = sum_u m_u[p]*in[c,p+u] / sum_u m_u[p]        (zero padding)

Symmetry m_{-u}[p] = m_u[p-u] -> only 25 maps computed on an extended grid,
all 49 aligned maps then materialized by DMA remap (DMA can cross partitions;
compute engines cannot, so every row shift lives in a free dim).
"""

import numpy as np

B, C, H, W = 2, 32, 256, 256
CG = 3
R = 3                      # K//2
NB = 4                     # h-blocks per batch
RB = H // NB               # 64 out rows per core
NCORES = 8

GR = RB + 2 * R            # 70 rows   (out rows -3..66)
GX = W + 4 * R             # 268 guide cols (-6..261)
IX = W + 2 * R             # 262 input cols (-3..258)
MR = RB + R                # 67 map rows (-3..63)
MJ = W + 2 * R             # 262 map cols (-3..258)
MS = W + 2 * R             # 262 per-map stride in K25 (even)
WH = 2                     # w halves in apply layout
XC = W // WH               # 128
XW = XC + 2 * R            # 134 per-half x window

UPLUS = [(0, ux) for ux in range(0, R + 1)] + [
    (uy, ux) for uy in range(1, R + 1) for ux in range(-R, R + 1)
]
ALL_U = []
for uy in range(-R, R + 1):
    for ux in range(-R, R + 1):
        if (uy, ux) in UPLUS:
            ALL_U.append((uy, ux, UPLUS.index((uy, ux)), False))
        else:
            ALL_U.append((uy, ux, UPLUS.index((-uy, -ux)), True))

_COMPILED = None


def _build_nc(legalize=True):
    import concourse.bass as bass
    import concourse.mybir as mybir
    from concourse.bass import AP
    from concourse.tile import TileContext

    fp32 = mybir.dt.float32
    fp16 = mybir.dt.float16
    ALU = mybir.AluOpType
    ACTF = mybir.ActivationFunctionType

    nc = bass.Bass()

    guide_d = nc.declare_dram_parameter("guide", [CG, GR, GX], fp32, isOutput=False)
    inp_d = nc.declare_dram_parameter("inp", [C, GR, IX], fp32, isOutput=False)
    rr_d = nc.declare_dram_parameter("rr25", [1, 25], fp32, isOutput=False)
    sig_d = nc.declare_dram_parameter("sigma", [1, 1], fp32, isOutput=False)
    ident_d = nc.declare_dram_parameter("ident", [128, 128], fp16, isOutput=False)
    out_d = nc.declare_dram_parameter("out", [C, RB, W], fp32, isOutput=True)

    def sb(t, p0, pn, off, dims):
        """AP into sbuf/psum tensor: partitions [p0, p0+pn), free offset+dims."""
        sl = t[p0:p0 + pn]
        return AP(sl.tensor, sl.offset + off, [sl.ap[0], *dims])

    def dr_ap(d, off, dims):
        full = d[:]
        return AP(full.tensor, full.offset + off, dims)

    from contextlib import ExitStack

    with TileContext(nc) as tc, ExitStack() as es:
        def mk(name, shape, dt):
            return es.enter_context(nc.sbuf_tensor(name, shape, dt))

        # aliasing region: guide-phase scratch (G4/D3B/SREDG/INH) is dead by
        # the time INB7O (odd-parity input copy) is written; OverlapTracker
        # fences reads->writes by byte range.
        _base0 = ((nc.sbuf_base + 31) // 32) * 32
        _ARENA_BYTES = 190 * 1024
        es.enter_context(nc.sbuf_tensor("ARENA", [128, _ARENA_BYTES], mybir.dt.uint8))
        _off = [_base0]
        def at(name, shape, dt, offset=None):
            if offset is None:
                offset = _off[0]
            import functools, operator
            sz = functools.reduce(operator.mul, shape[1:]) * mybir.dt.size(dt)
            h = nc.alloc_sbuf_tensor_at(name, shape, dt, offset=offset, align_bytes=32)
            _off[0] = max(_off[0], offset + ((sz + 31) // 32) * 32)
            return h

        INB7 = at("INB7", [128, C * 7 * XW], fp16)          # (c,dr,x)
        _arena = _off[0]
        INB7O = at("INB7O", [128, C * 7 * XW], fp16, _arena)
        # guide-phase scratch aliases INB7O's bytes (dead before INB7O written)
        def _al(n):
            return ((n + 31) // 32) * 32
        NSLOT = 6
        _g4b = _al(4 * CG * GX * 4)
        _d3b = _al(NSLOT * CG * MJ * 4)
        _sqb = _al(NSLOT * CG * MJ * 4)
        _srb = _al(NSLOT * MJ * 4)
        G4 = at("G4", [128, 4 * CG * GX], fp32, _arena)
        D3R = at("D3R", [128, NSLOT * CG * MJ], fp32, _arena + _g4b)
        SQR = at("SQR", [128, NSLOT * CG * MJ], fp32, _arena + _g4b + _d3b)
        SRR = at("SRR", [128, NSLOT * MJ], fp32, _arena + _g4b + _d3b + _sqb)
        assert _g4b + _d3b + _sqb + _srb <= C * 7 * XW * 2, (
            _g4b + _d3b + _sqb + _srb, C * 7 * XW * 2)
        _off[0] = _arena + ((C * 7 * XW * 2 + 31) // 32) * 32
        K25 = at("K25", [128, 25 * MS], fp16)               # (m,j)
        KA = at("KA", [128, 49 * XC], fp16)                 # (u,x)
        NORM = at("NORM", [128, XC], fp32)
        RCP = at("RCP", [128, XC], fp32)
        BROWT = at("BROWT", [128, 25], fp32)
        IDENT = at("IDENT", [128, 128], fp16)
        PBUFS = [at(f"P{i}T", [128, 4096], fp16) for i in range(4)]
        OUTC = at("OUTC", [128, 2 * 8 * XC], fp32)
        BROW = at("BROW", [1, 25], fp32)
        SIG = at("SIG", [1, 1], fp32)
        SIG2 = at("SIG2", [1, 1], fp32)
        IS2 = at("IS2", [1, 1], fp32)
        RR = at("RR", [1, 25], fp32)
        assert _off[0] <= _base0 + _ARENA_BYTES, (_off[0], _base0)
        ACC = es.enter_context(nc.psum_tensor("ACC", [128, 4096], fp32))
        if True:
            v, s, g, t, sync = nc.vector, nc.scalar, nc.gpsimd, nc.tensor, nc.sync

            # ---- guide load first (gates the first subs) ----
            for dy in range(4):
                dst = sb(G4, 0, MR, dy * (CG * GX), [[GX, CG], [1, GX]])
                src = dr_ap(guide_d, dy * GX, [[GX, MR], [GR * GX, CG], [1, GX]])
                sync.dma_start(out=dst, in_=src)

            # ---- constants ----
            sync.dma_start(out=RR[:], in_=rr_d[:])
            sync.dma_start(out=SIG[:], in_=sig_d[:])
            sync.dma_start(out=IDENT[:], in_=ident_d[:])
            v.tensor_tensor(out=SIG2[:], in0=SIG[:], in1=SIG[:], op=ALU.mult)
            v.reciprocal(out=IS2[:], in_=SIG2[:])
            v.tensor_scalar(out=BROW[:], in0=RR[:], scalar1=IS2[0:1, 0:1],
                            scalar2=-0.5, op0=ALU.mult, op1=ALU.mult)
            # broadcast partition 0 -> all via DRAM round-trip (0-step read)
            wscr = nc.dram_tensor("wscr", [1, 25], fp32)
            sync.dma_start(out=wscr[:], in_=BROW[:])
            sync.dma_start(out=BROWT[:],
                           in_=dr_ap(wscr, 0, [[0, 128], [1, 25]]))

            # ---- stage fp16 input in DRAM (software-DGE cast DMA), then
            #      row-replicated loads straight from DRAM (overlapping reads).
            #      1 leading pad element so the x-1 (odd-parity) reads stay
            #      in bounds. ----
            INHD = nc.dram_tensor("INHD", [1, C * GR * IX + 2], fp16)
            g.dma_start(out=dr_ap(INHD, 1, [[1, C * GR * IX]]),
                        in_=dr_ap(inp_d, 0, [[1, C * GR * IX]]))
            # dr order matches apply-map order (m=0 is uy=0 -> dr3 first)
            DR_ORDER = [3, 4, 2, 5, 1, 6, 0]
            for dr in DR_ORDER:
                for wh in range(WH):
                    dst = sb(INB7, wh * 64, 64, dr * XW, [[7 * XW, C], [1, XW]])
                    src = dr_ap(INHD, 1 + dr * IX + wh * XC,
                                [[IX, 64], [GR * IX, C], [1, XW]])
                    sync.dma_start(out=dst, in_=src)

            # kA slot order: aligned maps at slot=m, shifted at 24+m
            ka_slot = {}
            for (uy, ux, m, shifted) in ALL_U:
                ka_slot[(uy, ux)] = m if not shifted else 24 + m
            # batch tables keyed by the last map index that completes them
            AL_CHUNKS = {}
            CH = 4
            for m0c in range(0, 25, CH):
                nmap = min(CH, 25 - m0c)
                AL_CHUNKS.setdefault(m0c + nmap - 1, []).append((m0c, nmap))
            SH_GROUPS = {}
            _vy_base = {0: 1, 1: 4, 2: 11, 3: 18}
            for (vy, mb, nmap) in [(0, 1, 3), (1, 4, 4), (1, 8, 3), (2, 11, 4),
                                   (2, 15, 3), (3, 18, 4), (3, 22, 3)]:
                vxmin = ((-3 if vy >= 1 else 1) + (mb - _vy_base[vy]))
                SH_GROUPS.setdefault(mb + nmap - 1, []).append((vy, mb, nmap, vxmin))

            # ---- guide phase: per-map pipeline (Pool sub -> ACT sq ->
            #      DVE c-reduce -> ACT exp w/ per-map spatial bias) ----
            for m, (uy, ux) in enumerate(UPLUS):
                sl = m % NSLOT
                in0 = sb(G4, 0, MR, uy * CG * GX + 3 + ux, [[GX, CG], [1, MJ]])
                in1 = sb(G4, 0, MR, 3, [[GX, CG], [1, MJ]])
                d3 = sb(D3R, 0, MR, sl * CG * MJ, [[MJ, CG], [1, MJ]])
                sub_eng = v if m < 9 else g
                sub_eng.tensor_tensor(out=d3, in0=in0, in1=in1, op=ALU.subtract)
                s.activation(out=sb(SQR, 0, MR, sl * CG * MJ, [[1, CG * MJ]]),
                             in_=sb(D3R, 0, MR, sl * CG * MJ, [[1, CG * MJ]]),
                             func=ACTF.Square)
                _red = v.tensor_reduce(out=sb(SRR, 0, MR, sl * MJ, [[1, MJ]]),
                                in_=sb(SQR, 0, MR, sl * CG * MJ,
                                       [[1, MJ], [MJ, CG]]),
                                axis=mybir.AxisListType.X, op=ALU.add)
                if m == 22:
                    last_reduce = _red
                s.activation(out=sb(K25, 0, MR, m * MS, [[1, MJ]]),
                             in_=sb(SRR, 0, MR, sl * MJ, [[1, MJ]]),
                             func=ACTF.Exp, scale=-0.5,
                             bias=BROWT[0:MR, m:m + 1])

                # kA remap DMAs for batches completed by this map
                for (m0c, nmap) in AL_CHUNKS.get(m, ()):
                    for wh in range(WH):
                        dst = sb(KA, wh * 64, 64, m0c * XC, [[XC, nmap], [1, XC]])
                        srcc = sb(K25, 3, 64, m0c * MS + wh * XC + 3,
                                  [[MS, nmap], [1, XC]])
                        s.dma_start(out=dst, in_=srcc)
                for (vy, mb, nmap, vxmin) in SH_GROUPS.get(m, ()):
                    for wh in range(WH):
                        dst = sb(KA, wh * 64, 64, (24 + mb) * XC,
                                 [[XC, nmap], [1, XC]])
                        srcc = sb(K25, 3 - vy, 64,
                                  mb * MS + wh * XC + 3 - vxmin,
                                  [[MS - 1, nmap], [1, XC]])
                        s.dma_start(out=dst, in_=srcc)

            # ---- odd-parity (x-1) copy, per dr-slice on ACT (4) + Pool (3)
            #      from INB7 in SBUF (keeps DMA engines free) ----
            for di, dr in enumerate(DR_ORDER):
                dst = sb(INB7O, 0, 128, dr * XW + 1, [[7 * XW, C], [1, XW - 1]])
                srcc = sb(INB7, 0, 128, dr * XW, [[7 * XW, C], [1, XW - 1]])
                if di % 2 == 0:
                    s.copy(out=dst, in_=srcc)
                else:
                    g.tensor_copy(dst, srcc)

            # ---- apply: 49 offsets; even-parity first, by map readiness ----
            evens = sorted((e for e in ALL_U if (3 + e[1]) % 2 == 0),
                           key=lambda e: e[2])
            odds = sorted((e for e in ALL_U if (3 + e[1]) % 2 == 1),
                          key=lambda e: e[2])
            order = evens + odds
            first = True
            for oi, ent in enumerate(order):
                uy, ux, m, shifted = ent
                ui = ka_slot[(uy, ux)]
                off = (uy + 3) * XW + 3 + ux
                if (3 + ux) % 2 == 0:
                    in0 = sb(INB7, 0, 128, off, [[7 * XW, C], [1, XC]])
                else:
                    in0 = sb(INB7O, 0, 128, off + 1, [[7 * XW, C], [1, XC]])
                in1 = sb(KA, 0, 128, ui * XC, [[0, C], [1, XC]])
                P = PBUFS[oi % 4]
                _tt = v.tensor_tensor(out=sb(P, 0, 128, 0, [[XC, C], [1, XC]]),
                                      in0=in0, in1=in1, op=ALU.mult)
                if oi < 4:
                    from concourse.tile import add_dep_helper
                    add_dep_helper(_tt.ins, last_reduce.ins, sync=False,
                                   reason="hold applies until guide done (PE warmup)")
                last = oi == len(order) - 1
                for bk in range(8):
                    t.matmul(ACC[:, bk * 512:(bk + 1) * 512], IDENT[:, :],
                             P[:, bk * 512:(bk + 1) * 512], start=first, stop=last)
                first = False

            # ---- norm (Pool TT accumulate chain) + reciprocal (DVE);
            #      emitted late so the scheduler orders recip after the odd
            #      products in DVE's in-order stream ----
            g.memset(NORM[:, :], 0.0)
            for u in range(49):
                g.tensor_tensor(out=NORM[:, :],
                                in0=sb(KA, 0, 128, u * XC, [[1, XC]]),
                                in1=NORM[:, :], op=ALU.add)
            v.reciprocal(out=RCP[:, :], in_=NORM[:, :])

            # ---- finish: out = acc * rcp (bcast over c), 4 chunks of 8 ch ----
            for ch in range(4):
                obuf = (ch % 2) * 8 * XC
                a_sl = ACC[:, ch * 1024:(ch + 1) * 1024]
                a_ap = AP(a_sl.tensor, a_sl.offset, [a_sl.ap[0], [XC, 8], [1, XC]])
                r_ap = sb(RCP, 0, 128, 0, [[0, 8], [1, XC]])
                o_ap = sb(OUTC, 0, 128, obuf, [[XC, 8], [1, XC]])
                v.tensor_tensor(out=o_ap, in0=a_ap, in1=r_ap, op=ALU.mult)
                for wh in range(WH):
                    srcc = sb(OUTC, wh * 64, 64, obuf, [[XC, 8], [1, XC]])
                    dst = dr_ap(out_d, ch * 8 * RB * W + wh * XC,
                                [[W, 64], [RB * W, 8], [1, XC]])
                    sync.dma_start(out=dst, in_=srcc)

    if legalize:
        _legalize_waits(nc)
    return nc


def _legalize_waits(nc):
    """walrus codegen allows 1 sem-wait on DMA instructions (2 elsewhere);
    Tile can emit more. Move excess waits onto InstEventSemaphore nops
    inserted just before, on the same engine (sequencer stalls, then issues)."""
    import concourse.mybir as mybir

    ctr = [0]
    for bb in nc.main_func.blocks:
        out = []
        changed = False
        for ins in bb.instructions:
            cap = 1
            si = ins.sync_info
            waits = list(si.on_wait) if si is not None else []
            if len(waits) > cap:
                keep = waits[:cap]
                extra = waits[cap:]
                while extra:
                    chunk, extra = extra[:1], extra[1:]
                    e = mybir.InstEventSemaphore(
                        name=f"wsplit-{ctr[0]}", ins=[], outs=[])
                    ctr[0] += 1
                    e.engine = ins.engine
                    e.sync_info = mybir.SyncInfo(on_wait=chunk, on_update=[])
                    out.append(e)
                ins.sync_info = mybir.SyncInfo(on_wait=keep, on_update=list(si.on_update))
                changed = True
            out.append(ins)
        if changed:
            bb.instructions = out
    return nc


def _host_prep(input, input_for_kernel, sigma_for_kernel):
    inp = np.asarray(input, dtype=np.float32)
    gui = np.asarray(input_for_kernel, dtype=np.float32)
    sig = np.float32(np.asarray(sigma_for_kernel).reshape(()))

    # pad rows/cols by 6 each side (covers all slice windows with zeros)
    gp = np.zeros((B, CG, H + 12, W + 12), dtype=np.float32)
    gp[:, :, 6:6 + H, 6:6 + W] = gui
    ip = np.zeros((B, C, H + 12, W + 12), dtype=np.float32)
    ip[:, :, 6:6 + H, 6:6 + W] = inp

    rr = np.array([[float(uy * uy + ux * ux) for (uy, ux) in UPLUS]],
                  dtype=np.float32)
    ident = np.eye(128, dtype=np.float16)
    sig_arr = np.array([[sig]], dtype=np.float32)

    in_maps = []
    for core in range(NCORES):
        b, hb = divmod(core, NB)
        r0 = hb * RB
        # guide rows r0-3..r0+66, cols -6..258 -> gp[rows 6+r0-3 .., cols 0:265]
        gs = gp[b, :, 3 + r0: 3 + r0 + GR, 0:GX]
        # input rows r0-3..r0+66, cols -3..258 -> ip cols 3:3+262
        is_ = ip[b, :, 3 + r0: 3 + r0 + GR, 3:3 + IX]
        in_maps.append({
            "guide": np.ascontiguousarray(gs),
            "inp": np.ascontiguousarray(is_),
            "rr25": rr,
            "sigma": sig_arr,
            "ident": ident,
        })
    return in_maps


def kernel(input, input_for_kernel, sigma_for_kernel):
    global _COMPILED
    from concourse.bass_utils import run_bass_kernel_spmd

    if _COMPILED is None:
        _COMPILED = _build_nc()
    nc = _COMPILED

    in_maps = _host_prep(input, input_for_kernel, sigma_for_kernel)
    res = run_bass_kernel_spmd(nc, in_maps, core_ids=list(range(NCORES)))
    out = np.zeros((B, C, H, W), dtype=np.float32)
    for core in range(NCORES):
        b, hb = divmod(core, NB)
        out[b, :, hb * RB:(hb + 1) * RB, :] = res.results[core]["out"]
    return out



# revision 50
# speedup vs baseline: 2.3823x; 2.3823x over previous
"""Bilateral filter (K=7, guide channels=3) Trainium2 Bass kernel — v9.

Contract: kernel(**inputs) takes FULL unsharded numpy inputs
(input [2,32,256,256] f32, input_for_kernel [2,3,256,256] f32,
sigma_for_kernel scalar f32) and returns the full output [2,32,256,256] f32.
Shards internally over 8 NeuronCores: (batch=2) x (4 h-blocks of 64 rows).

Math (identical to the reference up to fp rounding; the spatial-gaussian
normalization cancels in ker/norm):
  m_u[p]   = exp(-0.5*sum_c (g[c,p+u]-g[c,p])^2) * exp(-0.5*(uy^2+ux^2)/s^2)
  out[c,p] = sum_u m_u[p]*in[c,p+u] / sum_u m_u[p]        (zero padding)

Symmetry m_{-u}[p] = m_u[p-u] -> only 25 maps computed on an extended grid,
all 49 aligned maps then materialized by DMA remap.

Engine plan (v9): DVE runs 47 of the 49 apply products (fp16 TT at 2
elem/cyc) plus the early c-reduce adds; Pool runs guide subs, late
c-reduce adds, 2 late products (dedicated buffers) and the norm tree;
ACT runs squares+exps only; PE accumulates all products into PSUM via
identity matmuls; both input alignment parities are prepared host-side
and loaded with one DMA per (dr,half).
"""

import numpy as np

B, C, H, W = 2, 32, 256, 256
CG = 3
R = 3                      # K//2
NB = 4                     # h-blocks per batch
RB = H // NB               # 64 out rows per core
NCORES = 8

GR = RB + 2 * R            # 70 rows   (out rows -3..66)
GX = W + 4 * R             # 268 guide cols (-6..261)
MR = RB + R                # 67 map rows (-3..63)
MJ = W + 2 * R             # 262 map cols (-3..258)
MS = W + 2 * R             # 262 per-map stride in K25 (even)
WH = 2                     # w halves in apply layout
XC = W // WH               # 128
XW = XC + 2 * R            # 134 per-half x window

UPLUS = [(0, ux) for ux in range(0, R + 1)] + [
    (uy, ux) for uy in range(1, R + 1) for ux in range(-R, R + 1)
]
ALL_U = []
for uy in range(-R, R + 1):
    for ux in range(-R, R + 1):
        if (uy, ux) in UPLUS:
            ALL_U.append((uy, ux, UPLUS.index((uy, ux)), False))
        else:
            ALL_U.append((uy, ux, UPLUS.index((-uy, -ux)), True))

N_POOL_PRODUCTS = 3        # late products on Pool (dedicated bufs, no reuse)

_COMPILED = None


def _build_nc(legalize=True):
    import concourse.bass as bass
    import concourse.mybir as mybir
    from concourse.bass import AP
    from concourse.tile import TileContext, add_dep_helper

    fp32 = mybir.dt.float32
    fp16 = mybir.dt.float16
    ALU = mybir.AluOpType
    ACTF = mybir.ActivationFunctionType

    nc = bass.Bass()

    guide_d = nc.declare_dram_parameter("guide16", [CG, GR, GX], fp16, isOutput=False)
    inp2_d = nc.declare_dram_parameter("inp2", [2 * 128, 7 * C * XW], fp16,
                                       isOutput=False)
    browt_d = nc.declare_dram_parameter("browt", [128, 25], fp32, isOutput=False)
    ident_d = nc.declare_dram_parameter("ident", [128, 128], fp16, isOutput=False)
    out_d = nc.declare_dram_parameter("out", [C, RB, W], fp32, isOutput=True)

    def sb(t, p0, pn, off, dims):
        """AP into sbuf/psum tensor: partitions [p0, p0+pn), free offset+dims."""
        sl = t[p0:p0 + pn]
        return AP(sl.tensor, sl.offset + off, [sl.ap[0], *dims])

    def dr_ap(d, off, dims):
        full = d[:]
        return AP(full.tensor, full.offset + off, dims)

    from contextlib import ExitStack

    NSLOT = 4
    CXW = C * XW

    with TileContext(nc) as tc, ExitStack() as es:
        def mk(name, shape, dt):
            return es.enter_context(nc.sbuf_tensor(name, shape, dt))

        # both alignment parities in one tensor: even at 0, odd at 7*CXW
        INBB = mk("INBB", [128, 14 * CXW], fp16)     # (par,dr,c,x)
        G4 = mk("G4", [128, 4 * CG * GX], fp16)      # (dy,c,x)
        D3 = mk("D3", [128, 4096], fp16)   # padded: doubles as 4th pool product buf
        SQ = mk("SQ", [128, NSLOT * CG * MJ], fp16)
        T2 = mk("T2", [128, NSLOT * MJ], fp16)
        K25 = mk("K25", [128, 25 * MS], fp16)        # (m,j)
        KA = mk("KA", [128, 49 * XC], fp16)          # (u,x)
        NT = G4                                      # norm tree scratch (G4
        # is dead once the last sub has read it; OverlapTracker fences)
        NORM = mk("NORM", [128, XC], fp32)
        RCP = mk("RCP", [128, XC], fp32)
        BROWT = mk("BROWT", [128, 25], fp32)
        IDENT = mk("IDENT", [128, 128], fp16)
        PBUFS = [mk(f"P{i}T", [128, 4096], fp16) for i in range(3)]
        POOLB = [mk("PL0T", [128, 4096], fp16),
                 mk("PL1T", [128, 4096], fp16)]
        POOLB.append(K25)                            # 3rd pool product buf
        OUTC = K25.bitcast(fp32)                     # finish bufs alias K25
        OUTG = G4.bitcast(fp32)                      # ...and G4 (chunk 3);
        # both are dead long before the finish runs (OverlapTracker fences)
        ACC = es.enter_context(nc.psum_tensor("ACC", [128, 4096], fp32))

        v, s, g, t, sync = nc.vector, nc.scalar, nc.gpsimd, nc.tensor, nc.sync

        # ---- guide load first, issued from the ACT queue (idle early, and
        #      its own first consumer is the first square anyway) ----
        for dy in range(4):
            dst = sb(G4, 0, MR, dy * (CG * GX), [[GX, CG], [1, GX]])
            src = dr_ap(guide_d, dy * GX, [[GX, MR], [GR * GX, CG], [1, GX]])
            s.dma_start(out=dst, in_=src)

        # ---- input loads: one DMA per (dr, half) covering both parities ----
        def load_in(dr, half):
            p0 = half * 64
            dst = sb(INBB, p0, 64, dr * CXW, [[7 * CXW, 2], [1, CXW]])
            src = dr_ap(inp2_d, p0 * 7 * CXW + dr * CXW,
                        [[7 * CXW, 64], [128 * 7 * CXW, 2], [1, CXW]])
            sync.dma_start(out=dst, in_=src)

        # ---- constants (BROWT precomputed host-side from sigma) ----
        sync.dma_start(out=BROWT[:], in_=browt_d[:])
        load_in(3, 0)
        load_in(3, 1)
        sync.dma_start(out=IDENT[:], in_=ident_d[:])
        # remaining loads emitted at these map iterations (both halves):
        LOAD_AT = {2: 4, 6: 2, 10: 5, 12: 1, 16: 6, 18: 0}

        # kA slot order: aligned maps at slot=m, shifted at 24+m
        ka_slot = {}
        for (uy, ux, m, shifted) in ALL_U:
            ka_slot[(uy, ux)] = m if not shifted else 24 + m
        # remap batches keyed by the last map index that completes them;
        # first chunk split (0-1 / 2-3) so products start earlier
        AL_CHUNKS = {}
        CH = 4
        _al_list = [(0, 2), (2, 2)] + [(m0c, min(CH, 25 - m0c))
                                       for m0c in range(CH, 25, CH)]
        for (m0c, nmap) in _al_list:
            AL_CHUNKS.setdefault(m0c + nmap - 1, []).append((m0c, nmap))
        _sh_list = [(0, 1, 3), (1, 4, 4), (1, 8, 3), (2, 11, 4),
                    (2, 15, 3), (3, 18, 4), (3, 22, 3)]
        SH_GROUPS = {}
        _vy_base = {0: 1, 1: 4, 2: 11, 3: 18}
        for (vy, mb, nmap) in _sh_list:
            vxmin = ((-3 if vy >= 1 else 1) + (mb - _vy_base[vy]))
            SH_GROUPS.setdefault(mb + nmap - 1, []).append((vy, mb, nmap, vxmin))

        _al_key = {}
        for (m0c, nmap) in _al_list:
            for mm in range(m0c, m0c + nmap):
                _al_key[mm] = m0c + nmap - 1
        _sh_key = {}
        for (vy, mb, nmap) in _sh_list:
            for mm in range(mb, mb + nmap):
                _sh_key[mm] = mb + nmap - 1

        def ready_at(e):
            uy, ux, m, shifted = e
            k = _al_key[m] if not shifted else _sh_key[m]
            if uy == 2 and k == 11:
                k = 14   # defer dr5 so its input load leaves the early DMA
                         # window to the latency-critical dr3/dr4/dr2 loads
            return k

        # products ordered by readiness; N_POOL_PRODUCTS mid-ready ones are
        # deferred to Pool (emitted after the map loop -> they sit in Pool's
        # stream after its guide work, filling its idle tail)
        order = sorted(ALL_U, key=lambda e: (ready_at(e), e[3], e[2], (3 + e[1]) % 2))
        mid = [e for e in order if 15 <= ready_at(e) <= 19]
        pool_ents = mid[:N_POOL_PRODUCTS]
        pool_set = set(id(e) for e in pool_ents)
        prod_by_map = {}
        for e in order:
            if id(e) not in pool_set:
                prod_by_map.setdefault(ready_at(e), []).append(e)

        deferred = []
        pidx = [0]        # DVE products emitted (PBUF ring index)
        nemit = [0]       # all products emitted
        flushed = [-1]
        exp_ins = {}

        def emit_product(ent):
            uy, ux, m, shifted = ent
            ui = ka_slot[(uy, ux)]
            dr = uy + 3
            par = (3 + ux) % 2       # 1 -> odd base copy
            base = par * 7 * CXW + dr * CXW + (3 + ux) + par
            first = nemit[0] == 0
            last = nemit[0] == 48
            if id(ent) in pool_set:
                P = POOLB[nemit[0] - pidx[0]]
                eng = g
            else:
                P = PBUFS[pidx[0] % 3]
                eng = v
                pidx[0] += 1
            if not last:
                in0 = sb(INBB, 0, 128, base, [[XW, C], [1, XC]])
                in1 = sb(KA, 0, 128, ui * XC, [[0, C], [1, XC]])
                eng.tensor_tensor(out=sb(P, 0, 128, 0, [[XC, C], [1, XC]]),
                                  in0=in0, in1=in1, op=ALU.mult)
                for bk in range(8):
                    t.matmul(ACC[:, bk * 512:(bk + 1) * 512], IDENT[:, :],
                             P[:, bk * 512:(bk + 1) * 512],
                             start=first, stop=False)
            else:
                # split the last product into 4 bank-pair chunks (8 channels
                # each) so the finish TTs pipeline with PE's final matmuls
                for ch in range(4):
                    in0 = sb(INBB, 0, 128, base + 8 * ch * XW,
                             [[XW, 8], [1, XC]])
                    in1 = sb(KA, 0, 128, ui * XC, [[0, 8], [1, XC]])
                    eng.tensor_tensor(
                        out=sb(P, 0, 128, 8 * ch * XC, [[XC, 8], [1, XC]]),
                        in0=in0, in1=in1, op=ALU.mult)
                    for bk in (2 * ch, 2 * ch + 1):
                        t.matmul(ACC[:, bk * 512:(bk + 1) * 512], IDENT[:, :],
                                 P[:, bk * 512:(bk + 1) * 512],
                                 start=False, stop=True)
            nemit[0] += 1

        def flush_pair(m0, nb, emit_products=True):
            """c-reduce adds (batched over nb contiguous slots), exps, then
            remap DMAs + dependent products for all newly-unlocked keys.
            Early pairs reduce on DVE (still lightly loaded); later ones on
            Pool so the map pipeline never queues behind DVE products."""
            mlast = m0 + nb - 1
            s0 = m0 % NSLOT
            red = v if mlast <= 1 else g
            red.tensor_tensor(
                out=sb(T2, 0, MR, s0 * MJ, [[MJ, nb], [1, MJ]]),
                in0=sb(SQ, 0, MR, s0 * CG * MJ, [[CG * MJ, nb], [1, MJ]]),
                in1=sb(SQ, 0, MR, s0 * CG * MJ + MJ,
                       [[CG * MJ, nb], [1, MJ]]),
                op=ALU.add)
            red.tensor_tensor(
                out=sb(T2, 0, MR, s0 * MJ, [[MJ, nb], [1, MJ]]),
                in0=sb(T2, 0, MR, s0 * MJ, [[MJ, nb], [1, MJ]]),
                in1=sb(SQ, 0, MR, s0 * CG * MJ + 2 * MJ,
                       [[CG * MJ, nb], [1, MJ]]),
                op=ALU.add)
            for mm in range(m0, mlast + 1):
                exp_ins[mm] = s.activation(
                    out=sb(K25, 0, MR, mm * MS, [[1, MJ]]),
                    in_=sb(T2, 0, MR, (mm % NSLOT) * MJ, [[1, MJ]]),
                    func=ACTF.Exp, scale=-0.5,
                    bias=BROWT[0:MR, mm:mm + 1])
            for mk in range(flushed[0] + 1, mlast + 1):
                for (m0c, nmap) in AL_CHUNKS.get(mk, ()):
                    for wh in range(WH):
                        dst = sb(KA, wh * 64, 64, m0c * XC,
                                 [[XC, nmap], [1, XC]])
                        srcc = sb(K25, 3, 64, m0c * MS + wh * XC + 3,
                                  [[MS, nmap], [1, XC]])
                        sync.dma_start(out=dst, in_=srcc)
                for (vy, mb, nmap, vxmin) in SH_GROUPS.get(mk, ()):
                    for wh in range(WH):
                        dst = sb(KA, wh * 64, 64, (24 + mb) * XC,
                                 [[XC, nmap], [1, XC]])
                        srcc = sb(K25, 3 - vy, 64,
                                  mb * MS + wh * XC + 3 - vxmin,
                                  [[MS - 1, nmap], [1, XC]])
                        sync.dma_start(out=dst, in_=srcc)
            for mk in range(flushed[0] + 1, mlast + 1):
                for ent in prod_by_map.get(mk, ()):
                    if emit_products:
                        emit_product(ent)
                    else:
                        deferred.append(ent)
            flushed[0] = mlast

        # ---- guide phase: per-map pipeline (Pool sub -> ACT sq ->
        #      adds c-reduce (pair-batched, one iter delayed) -> ACT exp w/
        #      per-map spatial bias) ----
        for m, (uy, ux) in enumerate(UPLUS):
            # sub/sq first (the first three subs run on the still-idle DVE,
            # ahead of any products in its queue), then the pair flush (one
            # iteration late so the reducer never waits on this square),
            # then loads so remap DMAs aren't queued behind them
            sl = m % NSLOT
            in0 = sb(G4, 0, MR, uy * CG * GX + 3 + ux, [[GX, CG], [1, MJ]])
            in1 = sb(G4, 0, MR, 3, [[GX, CG], [1, MJ]])
            d3 = sb(D3, 0, MR, sl * CG * MJ, [[MJ, CG], [1, MJ]])
            sub_eng = v if m <= 2 else g
            sub_eng.tensor_tensor(out=d3, in0=in0, in1=in1, op=ALU.subtract)
            _sq = s.activation(out=sb(SQ, 0, MR, sl * CG * MJ, [[1, CG * MJ]]),
                               in_=sb(D3, 0, MR, sl * CG * MJ, [[1, CG * MJ]]),
                               func=ACTF.Square)
            if m - 6 in exp_ins:
                add_dep_helper(_sq.ins, exp_ins[m - 6].ins, sync=False,
                               reason="keep ACT exps ahead of later squares")

            if m >= 2 and m % 2 == 0:
                flush_pair(m - 2, 2)

            if m in LOAD_AT:
                load_in(LOAD_AT[m], 0)
                load_in(LOAD_AT[m], 1)
        flush_pair(22, 2)
        flush_pair(24, 1, emit_products=False)

        # ---- norm + recip emitted BEFORE the last products: Pool computes
        #      the norm right after its guide work, DVE reaches the recip
        #      while late products still stream, so the finish is gated only
        #      on the PSUM stop, not on a tail-end recip ----
        # KA has 49 slots: NT[0:24] = KA[0:24]+KA[24:48]; halve; add slot 48.
        g.tensor_tensor(out=sb(NT, 0, 128, 0, [[1, 24 * XC]]),
                        in0=sb(KA, 0, 128, 0, [[1, 24 * XC]]),
                        in1=sb(KA, 0, 128, 24 * XC, [[1, 24 * XC]]),
                        op=ALU.add)
        g.tensor_tensor(out=sb(NT, 0, 128, 0, [[1, 12 * XC]]),
                        in0=sb(NT, 0, 128, 0, [[1, 12 * XC]]),
                        in1=sb(NT, 0, 128, 12 * XC, [[1, 12 * XC]]),
                        op=ALU.add)
        g.tensor_tensor(out=sb(NT, 0, 128, 0, [[1, 6 * XC]]),
                        in0=sb(NT, 0, 128, 0, [[1, 6 * XC]]),
                        in1=sb(NT, 0, 128, 6 * XC, [[1, 6 * XC]]),
                        op=ALU.add)
        g.tensor_tensor(out=sb(NT, 0, 128, 0, [[1, 3 * XC]]),
                        in0=sb(NT, 0, 128, 0, [[1, 3 * XC]]),
                        in1=sb(NT, 0, 128, 3 * XC, [[1, 3 * XC]]),
                        op=ALU.add)
        g.tensor_tensor(out=sb(NT, 0, 128, 0, [[1, XC]]),
                        in0=sb(NT, 0, 128, 0, [[1, XC]]),
                        in1=sb(NT, 0, 128, XC, [[1, XC]]),
                        op=ALU.add)
        g.tensor_tensor(out=sb(NT, 0, 128, XC, [[1, XC]]),
                        in0=sb(NT, 0, 128, 2 * XC, [[1, XC]]),
                        in1=sb(KA, 0, 128, 48 * XC, [[1, XC]]),
                        op=ALU.add)
        g.tensor_tensor(out=NORM[:, :],
                        in0=sb(NT, 0, 128, 0, [[1, XC]]),
                        in1=sb(NT, 0, 128, XC, [[1, XC]]),
                        op=ALU.add)
        v.reciprocal(out=RCP[:, :], in_=NORM[:, :])

        # pool products (their matmuls sit mid-PE-queue), then the final
        # DVE products
        for ent in pool_ents:
            emit_product(ent)
        for ent in deferred:
            emit_product(ent)
        assert nemit[0] == 49

        # ---- finish: out = acc * rcp (bcast over c), 4 chunks of 8 ch,
        #      all on DVE (Pool cannot read PSUM) ----
        for ch in range(4):
            ot, obuf = (OUTC, ch * 8 * XC) if ch < 3 else (OUTG, 0)
            a_sl = ACC[:, ch * 1024:(ch + 1) * 1024]
            a_ap = AP(a_sl.tensor, a_sl.offset, [a_sl.ap[0], [XC, 8], [1, XC]])
            r_ap = sb(RCP, 0, 128, 0, [[0, 8], [1, XC]])
            o_ap = sb(ot, 0, 128, obuf, [[XC, 8], [1, XC]])
            v.tensor_tensor(out=o_ap, in0=a_ap, in1=r_ap, op=ALU.mult)
            for wh in range(WH):
                srcc = sb(ot, wh * 64, 64, obuf, [[XC, 8], [1, XC]])
                dst = dr_ap(out_d, ch * 8 * RB * W + wh * XC,
                            [[W, 64], [RB * W, 8], [1, XC]])
                sync.dma_start(out=dst, in_=srcc)

    if legalize:
        _legalize_waits(nc)
    return nc


def _legalize_waits(nc):
    """walrus codegen allows 1 sem-wait on DMA instructions (2 elsewhere);
    Tile can emit more. Move excess waits onto InstEventSemaphore nops
    inserted just before, on the same engine (sequencer stalls, then issues)."""
    import concourse.mybir as mybir

    ctr = [0]
    for bb in nc.main_func.blocks:
        out = []
        changed = False
        for ins in bb.instructions:
            cap = 1
            si = ins.sync_info
            waits = list(si.on_wait) if si is not None else []
            if len(waits) > cap:
                keep = waits[:cap]
                extra = waits[cap:]
                while extra:
                    chunk, extra = extra[:1], extra[1:]
                    e = mybir.InstEventSemaphore(
                        name=f"wsplit-{ctr[0]}", ins=[], outs=[])
                    ctr[0] += 1
                    e.engine = ins.engine
                    e.sync_info = mybir.SyncInfo(on_wait=chunk, on_update=[])
                    out.append(e)
                ins.sync_info = mybir.SyncInfo(on_wait=keep, on_update=list(si.on_update))
                changed = True
            out.append(ins)
        if changed:
            bb.instructions = out
    return nc


def _host_prep(input, input_for_kernel, sigma_for_kernel):
    inp = np.asarray(input, dtype=np.float32)
    gui = np.asarray(input_for_kernel, dtype=np.float32)
    sig = np.float32(np.asarray(sigma_for_kernel).reshape(()))

    # pad rows/cols by 6 each side (covers all slice windows with zeros)
    gp = np.zeros((B, CG, H + 12, W + 12), dtype=np.float16)
    gp[:, :, 6:6 + H, 6:6 + W] = gui
    ip = np.zeros((B, C, H + 12, W + 12), dtype=np.float16)
    ip[:, :, 6:6 + H, 6:6 + W] = inp

    rr = np.array([float(uy * uy + ux * ux) for (uy, ux) in UPLUS],
                  dtype=np.float32)
    browt = np.tile((-0.5 * rr / (sig * sig))[None, :], (128, 1)).astype(
        np.float32)
    ident = np.eye(128, dtype=np.float16)

    in_maps = []
    for core in range(NCORES):
        b, hb = divmod(core, NB)
        r0 = hb * RB
        # guide rows r0-3..r0+66, cols -6..261 -> gp[rows 6+r0-3 .., cols 0:GX]
        gs = gp[b, :, 3 + r0: 3 + r0 + GR, 0:GX]
        # input windows, both parities: even base x-3, odd base x-4
        # value[par, (wh,row), dr, c, x] = in[c, r0+row+dr-3, wh*128+x-3-par]
        inp2 = np.empty((2, 2, 64, 7, C, XW), dtype=np.float16)
        for par in range(2):
            for wh in range(WH):
                x0 = 6 + wh * XC - 3 - par
                for dr in range(7):
                    rlo = 6 + r0 + dr - 3
                    # [C, 64, XW] -> [64, C, XW]
                    inp2[par, wh, :, dr] = ip[b][:, rlo:rlo + 64,
                                                 x0:x0 + XW].transpose(1, 0, 2)
        in_maps.append({
            "guide16": np.ascontiguousarray(gs),
            "inp2": inp2.reshape(2 * 128, 7 * C * XW),
            "browt": browt,
            "ident": ident,
        })
    return in_maps


def kernel(input, input_for_kernel, sigma_for_kernel):
    global _COMPILED
    from concourse.bass_utils import run_bass_kernel_spmd

    if _COMPILED is None:
        _COMPILED = _build_nc()
    nc = _COMPILED

    in_maps = _host_prep(input, input_for_kernel, sigma_for_kernel)
    res = run_bass_kernel_spmd(nc, in_maps, core_ids=list(range(NCORES)))
    out = np.zeros((B, C, H, W), dtype=np.float32)
    for core in range(NCORES):
        b, hb = divmod(core, NB)
        out[b, :, hb * RB:(hb + 1) * RB, :] = res.results[core]["out"]
    return out


# revision 52
# speedup vs baseline: 2.6458x; 1.1106x over previous
"""Bilateral filter (K=7, guide channels=3) Trainium2 Bass kernel — v9.

Contract: kernel(**inputs) takes FULL unsharded numpy inputs
(input [2,32,256,256] f32, input_for_kernel [2,3,256,256] f32,
sigma_for_kernel scalar f32) and returns the full output [2,32,256,256] f32.
Shards internally over 8 NeuronCores: (batch=2) x (4 h-blocks of 64 rows).

Math (identical to the reference up to fp rounding; the spatial-gaussian
normalization cancels in ker/norm):
  m_u[p]   = exp(-0.5*sum_c (g[c,p+u]-g[c,p])^2) * exp(-0.5*(uy^2+ux^2)/s^2)
  out[c,p] = sum_u m_u[p]*in[c,p+u] / sum_u m_u[p]        (zero padding)

Symmetry m_{-u}[p] = m_u[p-u] -> only 25 maps computed on an extended grid,
all 49 aligned maps then materialized by DMA remap.

Engine plan (v9): DVE runs 47 of the 49 apply products (fp16 TT at 2
elem/cyc) plus the early c-reduce adds; Pool runs guide subs, late
c-reduce adds, 2 late products (dedicated buffers) and the norm tree;
ACT runs squares+exps only; PE accumulates all products into PSUM via
identity matmuls; both input alignment parities are prepared host-side
and loaded with one DMA per (dr,half).
"""

import numpy as np

B, C, H, W = 2, 32, 256, 256
CG = 3
R = 3                      # K//2
NB = 4                     # h-blocks per batch
RB = H // NB               # 64 out rows per core
NCORES = 8

GR = RB + 2 * R            # 70 rows   (out rows -3..66)
GX = W + 4 * R             # 268 guide cols (-6..261)
MR = RB + R                # 67 map rows (-3..63)
MJ = W + 2 * R             # 262 map cols (-3..258)
MS = W + 2 * R             # 262 per-map stride in K25 (even)
WH = 2                     # w halves in apply layout
XC = W // WH               # 128
XW = XC + 2 * R            # 134 per-half x window

UPLUS = [(0, ux) for ux in range(0, R + 1)] + [
    (uy, ux) for uy in range(1, R + 1) for ux in range(-R, R + 1)
]
ALL_U = []
for uy in range(-R, R + 1):
    for ux in range(-R, R + 1):
        if (uy, ux) in UPLUS:
            ALL_U.append((uy, ux, UPLUS.index((uy, ux)), False))
        else:
            ALL_U.append((uy, ux, UPLUS.index((-uy, -ux)), True))

N_POOL_PRODUCTS = 3        # late products on Pool (dedicated bufs, no reuse)

_COMPILED = None


def _build_nc(legalize=True):
    import concourse.bass as bass
    import concourse.mybir as mybir
    from concourse.bass import AP
    from concourse.tile import TileContext, add_dep_helper

    fp32 = mybir.dt.float32
    fp16 = mybir.dt.float16
    ALU = mybir.AluOpType
    ACTF = mybir.ActivationFunctionType

    nc = bass.Bass()

    guide_d = nc.declare_dram_parameter("guide16", [CG, GR, GX], fp16, isOutput=False)
    inp2_d = nc.declare_dram_parameter("inp2", [2 * 128, 7 * C * XW], fp16,
                                       isOutput=False)
    browt_d = nc.declare_dram_parameter("browt", [128, 25], fp32, isOutput=False)
    ident_d = nc.declare_dram_parameter("ident", [128, 128], fp16, isOutput=False)
    out_d = nc.declare_dram_parameter("out", [C, RB, W], fp32, isOutput=True)

    def sb(t, p0, pn, off, dims):
        """AP into sbuf/psum tensor: partitions [p0, p0+pn), free offset+dims."""
        sl = t[p0:p0 + pn]
        return AP(sl.tensor, sl.offset + off, [sl.ap[0], *dims])

    def dr_ap(d, off, dims):
        full = d[:]
        return AP(full.tensor, full.offset + off, dims)

    from contextlib import ExitStack

    NSLOT = 4
    CXW = C * XW

    with TileContext(nc) as tc, ExitStack() as es:
        def mk(name, shape, dt):
            return es.enter_context(nc.sbuf_tensor(name, shape, dt))

        # both alignment parities in one tensor: even at 0, odd at 7*CXW
        INBB = mk("INBB", [128, 14 * CXW], fp16)     # (par,dr,c,x)
        G4 = mk("G4", [128, 4 * CG * GX], fp16)      # (dy,c,x)
        D3 = mk("D3", [128, 4096], fp16)   # padded: doubles as 4th pool product buf
        SQ = mk("SQ", [128, NSLOT * CG * MJ], fp16)
        T2 = mk("T2", [128, NSLOT * MJ], fp16)
        K25 = mk("K25", [128, 25 * MS], fp16)        # (m,j)
        KA = mk("KA", [128, 49 * XC], fp16)          # (u,x)
        NT = G4                                      # norm tree scratch (G4
        # is dead once the last sub has read it; OverlapTracker fences)
        NORM = mk("NORM", [128, XC], fp32)
        RCP = mk("RCP", [128, XC], fp32)
        BROWT = mk("BROWT", [128, 25], fp32)
        IDENT = mk("IDENT", [128, 128], fp16)
        PBUFS = [mk(f"P{i}T", [128, 4096], fp16) for i in range(3)]
        POOLB = [mk("PL0T", [128, 4096], fp16),
                 mk("PL1T", [128, 4096], fp16)]
        POOLB.append(K25)                            # 3rd pool product buf
        OUTC = K25.bitcast(fp32)                     # finish bufs alias K25
        OUTG = G4.bitcast(fp32)                      # ...and G4 (chunk 3);
        # both are dead long before the finish runs (OverlapTracker fences)
        ACC = es.enter_context(nc.psum_tensor("ACC", [128, 4096], fp32))

        v, s, g, t, sync = nc.vector, nc.scalar, nc.gpsimd, nc.tensor, nc.sync

        # ---- guide load first, issued from the ACT queue (idle early, and
        #      its own first consumer is the first square anyway) ----
        for dy in range(4):
            dst = sb(G4, 0, MR, dy * (CG * GX), [[GX, CG], [1, GX]])
            src = dr_ap(guide_d, dy * GX, [[GX, MR], [GR * GX, CG], [1, GX]])
            s.dma_start(out=dst, in_=src)

        # ---- input loads: one DMA per (dr, half) covering both parities ----
        def load_in(dr, half):
            p0 = half * 64
            dst = sb(INBB, p0, 64, dr * CXW, [[7 * CXW, 2], [1, CXW]])
            src = dr_ap(inp2_d, p0 * 7 * CXW + dr * CXW,
                        [[7 * CXW, 64], [128 * 7 * CXW, 2], [1, CXW]])
            sync.dma_start(out=dst, in_=src)

        # ---- constants (BROWT precomputed host-side from sigma) ----
        sync.dma_start(out=BROWT[:], in_=browt_d[:])
        load_in(3, 0)
        load_in(3, 1)
        sync.dma_start(out=IDENT[:], in_=ident_d[:])
        # remaining loads emitted at these map iterations (both halves):
        LOAD_AT = {2: 4, 6: 2, 10: 5, 12: 1, 16: 6, 18: 0}

        # kA slot order: aligned maps at slot=m, shifted at 24+m
        ka_slot = {}
        for (uy, ux, m, shifted) in ALL_U:
            ka_slot[(uy, ux)] = m if not shifted else 24 + m
        # remap batches keyed by the last map index that completes them;
        # first chunk split (0-1 / 2-3) so products start earlier
        AL_CHUNKS = {}
        CH = 4
        _al_list = [(0, 1), (1, 1), (2, 2)] + [(m0c, min(CH, 25 - m0c))
                                       for m0c in range(CH, 25, CH)]
        for (m0c, nmap) in _al_list:
            AL_CHUNKS.setdefault(m0c + nmap - 1, []).append((m0c, nmap))
        _sh_list = [(0, 1, 3), (1, 4, 4), (1, 8, 3), (2, 11, 4),
                    (2, 15, 3), (3, 18, 4), (3, 22, 3)]
        SH_GROUPS = {}
        _vy_base = {0: 1, 1: 4, 2: 11, 3: 18}
        for (vy, mb, nmap) in _sh_list:
            vxmin = ((-3 if vy >= 1 else 1) + (mb - _vy_base[vy]))
            SH_GROUPS.setdefault(mb + nmap - 1, []).append((vy, mb, nmap, vxmin))

        _al_key = {}
        for (m0c, nmap) in _al_list:
            for mm in range(m0c, m0c + nmap):
                _al_key[mm] = m0c + nmap - 1
        _sh_key = {}
        for (vy, mb, nmap) in _sh_list:
            for mm in range(mb, mb + nmap):
                _sh_key[mm] = mb + nmap - 1

        def ready_at(e):
            uy, ux, m, shifted = e
            k = _al_key[m] if not shifted else _sh_key[m]
            if uy == 2 and k == 11:
                k = 14   # defer dr5 so its input load leaves the early DMA
                         # window to the latency-critical dr3/dr4/dr2 loads
            return k

        # products ordered by readiness; N_POOL_PRODUCTS mid-ready ones are
        # deferred to Pool (emitted after the map loop -> they sit in Pool's
        # stream after its guide work, filling its idle tail)
        order = sorted(ALL_U, key=lambda e: (ready_at(e), e[3], e[2], (3 + e[1]) % 2))
        mid = [e for e in order if 15 <= ready_at(e) <= 19]
        pool_ents = mid[:N_POOL_PRODUCTS]
        pool_set = set(id(e) for e in pool_ents)
        prod_by_map = {}
        for e in order:
            if id(e) not in pool_set:
                prod_by_map.setdefault(ready_at(e), []).append(e)

        deferred = []
        pidx = [0]        # DVE products emitted (PBUF ring index)
        nemit = [0]       # all products emitted
        flushed = [-1]
        exp_ins = {}

        def emit_product(ent):
            uy, ux, m, shifted = ent
            ui = ka_slot[(uy, ux)]
            dr = uy + 3
            par = (3 + ux) % 2       # 1 -> odd base copy
            base = par * 7 * CXW + dr * CXW + (3 + ux) + par
            first = nemit[0] == 0
            last = nemit[0] == 48
            if id(ent) in pool_set:
                P = POOLB[nemit[0] - pidx[0]]
                eng = g
            else:
                P = PBUFS[pidx[0] % 3]
                eng = v
                pidx[0] += 1
            if not last:
                in0 = sb(INBB, 0, 128, base, [[XW, C], [1, XC]])
                in1 = sb(KA, 0, 128, ui * XC, [[0, C], [1, XC]])
                eng.tensor_tensor(out=sb(P, 0, 128, 0, [[XC, C], [1, XC]]),
                                  in0=in0, in1=in1, op=ALU.mult)
                for bk in range(8):
                    t.matmul(ACC[:, bk * 512:(bk + 1) * 512], IDENT[:, :],
                             P[:, bk * 512:(bk + 1) * 512],
                             start=first, stop=False)
            else:
                # split the last product into 4 bank-pair chunks (8 channels
                # each) so the finish TTs pipeline with PE's final matmuls
                for ch in range(4):
                    in0 = sb(INBB, 0, 128, base + 8 * ch * XW,
                             [[XW, 8], [1, XC]])
                    in1 = sb(KA, 0, 128, ui * XC, [[0, 8], [1, XC]])
                    eng.tensor_tensor(
                        out=sb(P, 0, 128, 8 * ch * XC, [[XC, 8], [1, XC]]),
                        in0=in0, in1=in1, op=ALU.mult)
                    for bk in (2 * ch, 2 * ch + 1):
                        t.matmul(ACC[:, bk * 512:(bk + 1) * 512], IDENT[:, :],
                                 P[:, bk * 512:(bk + 1) * 512],
                                 start=False, stop=True)
            nemit[0] += 1

        def flush_pair(m0, nb, emit_products=True):
            """c-reduce adds (batched over nb contiguous slots), exps, then
            remap DMAs + dependent products for all newly-unlocked keys.
            Early pairs reduce on DVE (still lightly loaded); later ones on
            Pool so the map pipeline never queues behind DVE products."""
            mlast = m0 + nb - 1
            s0 = m0 % NSLOT
            red = v if mlast <= 1 else g
            red.tensor_tensor(
                out=sb(T2, 0, MR, s0 * MJ, [[MJ, nb], [1, MJ]]),
                in0=sb(SQ, 0, MR, s0 * CG * MJ, [[CG * MJ, nb], [1, MJ]]),
                in1=sb(SQ, 0, MR, s0 * CG * MJ + MJ,
                       [[CG * MJ, nb], [1, MJ]]),
                op=ALU.add)
            red.tensor_tensor(
                out=sb(T2, 0, MR, s0 * MJ, [[MJ, nb], [1, MJ]]),
                in0=sb(T2, 0, MR, s0 * MJ, [[MJ, nb], [1, MJ]]),
                in1=sb(SQ, 0, MR, s0 * CG * MJ + 2 * MJ,
                       [[CG * MJ, nb], [1, MJ]]),
                op=ALU.add)
            for mm in range(m0, mlast + 1):
                exp_ins[mm] = s.activation(
                    out=sb(K25, 0, MR, mm * MS, [[1, MJ]]),
                    in_=sb(T2, 0, MR, (mm % NSLOT) * MJ, [[1, MJ]]),
                    func=ACTF.Exp, scale=-0.5,
                    bias=BROWT[0:MR, mm:mm + 1])
            for mk in range(flushed[0] + 1, mlast + 1):
                for (m0c, nmap) in AL_CHUNKS.get(mk, ()):
                    for wh in range(WH):
                        dst = sb(KA, wh * 64, 64, m0c * XC,
                                 [[XC, nmap], [1, XC]])
                        srcc = sb(K25, 3, 64, m0c * MS + wh * XC + 3,
                                  [[MS, nmap], [1, XC]])
                        sync.dma_start(out=dst, in_=srcc)
                for (vy, mb, nmap, vxmin) in SH_GROUPS.get(mk, ()):
                    for wh in range(WH):
                        dst = sb(KA, wh * 64, 64, (24 + mb) * XC,
                                 [[XC, nmap], [1, XC]])
                        srcc = sb(K25, 3 - vy, 64,
                                  mb * MS + wh * XC + 3 - vxmin,
                                  [[MS - 1, nmap], [1, XC]])
                        sync.dma_start(out=dst, in_=srcc)
            for mk in range(flushed[0] + 1, mlast + 1):
                for ent in prod_by_map.get(mk, ()):
                    if emit_products:
                        emit_product(ent)
                    else:
                        deferred.append(ent)
            flushed[0] = mlast

        # ---- guide phase: per-map pipeline (Pool sub -> ACT sq ->
        #      adds c-reduce (pair-batched, one iter delayed) -> ACT exp w/
        #      per-map spatial bias) ----
        for m, (uy, ux) in enumerate(UPLUS):
            # sub/sq first (the first three subs run on the still-idle DVE,
            # ahead of any products in its queue), then the pair flush (one
            # iteration late so the reducer never waits on this square),
            # then loads so remap DMAs aren't queued behind them
            sl = m % NSLOT
            in0 = sb(G4, 0, MR, uy * CG * GX + 3 + ux, [[GX, CG], [1, MJ]])
            in1 = sb(G4, 0, MR, 3, [[GX, CG], [1, MJ]])
            d3 = sb(D3, 0, MR, sl * CG * MJ, [[MJ, CG], [1, MJ]])
            sub_eng = v if m <= 2 else g
            sub_eng.tensor_tensor(out=d3, in0=in0, in1=in1, op=ALU.subtract)
            _sq = s.activation(out=sb(SQ, 0, MR, sl * CG * MJ, [[1, CG * MJ]]),
                               in_=sb(D3, 0, MR, sl * CG * MJ, [[1, CG * MJ]]),
                               func=ACTF.Square)
            if m - 5 in exp_ins:
                add_dep_helper(_sq.ins, exp_ins[m - 5].ins, sync=False,
                               reason="keep ACT exps ahead of later squares")

            if m >= 2 and m % 2 == 0:
                flush_pair(m - 2, 2)

            if m in LOAD_AT:
                load_in(LOAD_AT[m], 0)
                load_in(LOAD_AT[m], 1)
        flush_pair(22, 2)
        flush_pair(24, 1, emit_products=False)

        # ---- norm + recip emitted BEFORE the last products: Pool computes
        #      the norm right after its guide work, DVE reaches the recip
        #      while late products still stream, so the finish is gated only
        #      on the PSUM stop, not on a tail-end recip ----
        # KA has 49 slots: NT[0:24] = KA[0:24]+KA[24:48]; halve; add slot 48.
        g.tensor_tensor(out=sb(NT, 0, 128, 0, [[1, 24 * XC]]),
                        in0=sb(KA, 0, 128, 0, [[1, 24 * XC]]),
                        in1=sb(KA, 0, 128, 24 * XC, [[1, 24 * XC]]),
                        op=ALU.add)
        g.tensor_tensor(out=sb(NT, 0, 128, 0, [[1, 12 * XC]]),
                        in0=sb(NT, 0, 128, 0, [[1, 12 * XC]]),
                        in1=sb(NT, 0, 128, 12 * XC, [[1, 12 * XC]]),
                        op=ALU.add)
        g.tensor_tensor(out=sb(NT, 0, 128, 0, [[1, 6 * XC]]),
                        in0=sb(NT, 0, 128, 0, [[1, 6 * XC]]),
                        in1=sb(NT, 0, 128, 6 * XC, [[1, 6 * XC]]),
                        op=ALU.add)
        g.tensor_tensor(out=sb(NT, 0, 128, 0, [[1, 3 * XC]]),
                        in0=sb(NT, 0, 128, 0, [[1, 3 * XC]]),
                        in1=sb(NT, 0, 128, 3 * XC, [[1, 3 * XC]]),
                        op=ALU.add)
        g.tensor_tensor(out=sb(NT, 0, 128, 0, [[1, XC]]),
                        in0=sb(NT, 0, 128, 0, [[1, XC]]),
                        in1=sb(NT, 0, 128, XC, [[1, XC]]),
                        op=ALU.add)
        g.tensor_tensor(out=sb(NT, 0, 128, XC, [[1, XC]]),
                        in0=sb(NT, 0, 128, 2 * XC, [[1, XC]]),
                        in1=sb(KA, 0, 128, 48 * XC, [[1, XC]]),
                        op=ALU.add)
        g.tensor_tensor(out=NORM[:, :],
                        in0=sb(NT, 0, 128, 0, [[1, XC]]),
                        in1=sb(NT, 0, 128, XC, [[1, XC]]),
                        op=ALU.add)
        v.reciprocal(out=RCP[:, :], in_=NORM[:, :])

        # pool products (their matmuls sit mid-PE-queue), then the final
        # DVE products
        for ent in pool_ents:
            emit_product(ent)
        for ent in deferred:
            emit_product(ent)
        assert nemit[0] == 49

        # ---- finish: out = acc * rcp (bcast over c), 4 chunks of 8 ch,
        #      all on DVE (Pool cannot read PSUM) ----
        for ch in range(4):
            ot, obuf = (OUTC, ch * 8 * XC) if ch < 3 else (OUTG, 0)
            a_sl = ACC[:, ch * 1024:(ch + 1) * 1024]
            a_ap = AP(a_sl.tensor, a_sl.offset, [a_sl.ap[0], [XC, 8], [1, XC]])
            r_ap = sb(RCP, 0, 128, 0, [[0, 8], [1, XC]])
            o_ap = sb(ot, 0, 128, obuf, [[XC, 8], [1, XC]])
            v.tensor_tensor(out=o_ap, in0=a_ap, in1=r_ap, op=ALU.mult)
            for wh in range(WH):
                srcc = sb(ot, wh * 64, 64, obuf, [[XC, 8], [1, XC]])
                dst = dr_ap(out_d, ch * 8 * RB * W + wh * XC,
                            [[W, 64], [RB * W, 8], [1, XC]])
                sync.dma_start(out=dst, in_=srcc)

    if legalize:
        _legalize_waits(nc)
    return nc


def _legalize_waits(nc):
    """walrus codegen allows 1 sem-wait on DMA instructions (2 elsewhere);
    Tile can emit more. Move excess waits onto InstEventSemaphore nops
    inserted just before, on the same engine (sequencer stalls, then issues)."""
    import concourse.mybir as mybir

    ctr = [0]
    for bb in nc.main_func.blocks:
        out = []
        changed = False
        for ins in bb.instructions:
            cap = 1
            si = ins.sync_info
            waits = list(si.on_wait) if si is not None else []
            if len(waits) > cap:
                keep = waits[:cap]
                extra = waits[cap:]
                while extra:
                    chunk, extra = extra[:1], extra[1:]
                    e = mybir.InstEventSemaphore(
                        name=f"wsplit-{ctr[0]}", ins=[], outs=[])
                    ctr[0] += 1
                    e.engine = ins.engine
                    e.sync_info = mybir.SyncInfo(on_wait=chunk, on_update=[])
                    out.append(e)
                ins.sync_info = mybir.SyncInfo(on_wait=keep, on_update=list(si.on_update))
                changed = True
            out.append(ins)
        if changed:
            bb.instructions = out
    return nc


def _host_prep(input, input_for_kernel, sigma_for_kernel):
    inp = np.asarray(input, dtype=np.float32)
    gui = np.asarray(input_for_kernel, dtype=np.float32)
    sig = np.float32(np.asarray(sigma_for_kernel).reshape(()))

    # pad rows/cols by 6 each side (covers all slice windows with zeros)
    gp = np.zeros((B, CG, H + 12, W + 12), dtype=np.float16)
    gp[:, :, 6:6 + H, 6:6 + W] = gui
    ip = np.zeros((B, C, H + 12, W + 12), dtype=np.float16)
    ip[:, :, 6:6 + H, 6:6 + W] = inp

    rr = np.array([float(uy * uy + ux * ux) for (uy, ux) in UPLUS],
                  dtype=np.float32)
    browt = np.tile((-0.5 * rr / (sig * sig))[None, :], (128, 1)).astype(
        np.float32)
    ident = np.eye(128, dtype=np.float16)

    in_maps = []
    for core in range(NCORES):
        b, hb = divmod(core, NB)
        r0 = hb * RB
        # guide rows r0-3..r0+66, cols -6..261 -> gp[rows 6+r0-3 .., cols 0:GX]
        gs = gp[b, :, 3 + r0: 3 + r0 + GR, 0:GX]
        # input windows, both parities: even base x-3, odd base x-4
        # value[par, (wh,row), dr, c, x] = in[c, r0+row+dr-3, wh*128+x-3-par]
        inp2 = np.empty((2, 2, 64, 7, C, XW), dtype=np.float16)
        for par in range(2):
            for wh in range(WH):
                x0 = 6 + wh * XC - 3 - par
                for dr in range(7):
                    rlo = 6 + r0 + dr - 3
                    # [C, 64, XW] -> [64, C, XW]
                    inp2[par, wh, :, dr] = ip[b][:, rlo:rlo + 64,
                                                 x0:x0 + XW].transpose(1, 0, 2)
        in_maps.append({
            "guide16": np.ascontiguousarray(gs),
            "inp2": inp2.reshape(2 * 128, 7 * C * XW),
            "browt": browt,
            "ident": ident,
        })
    return in_maps


def kernel(input, input_for_kernel, sigma_for_kernel):
    global _COMPILED
    from concourse.bass_utils import run_bass_kernel_spmd

    if _COMPILED is None:
        _COMPILED = _build_nc()
    nc = _COMPILED

    in_maps = _host_prep(input, input_for_kernel, sigma_for_kernel)
    res = run_bass_kernel_spmd(nc, in_maps, core_ids=list(range(NCORES)))
    out = np.zeros((B, C, H, W), dtype=np.float32)
    for core in range(NCORES):
        b, hb = divmod(core, NB)
        out[b, :, hb * RB:(hb + 1) * RB, :] = res.results[core]["out"]
    return out


# revision 56
# speedup vs baseline: 2.7354x; 1.0339x over previous
"""Bilateral filter (K=7, guide channels=3) Trainium2 Bass kernel — v9.

Contract: kernel(**inputs) takes FULL unsharded numpy inputs
(input [2,32,256,256] f32, input_for_kernel [2,3,256,256] f32,
sigma_for_kernel scalar f32) and returns the full output [2,32,256,256] f32.
Shards internally over 8 NeuronCores: (batch=2) x (4 h-blocks of 64 rows).

Math (identical to the reference up to fp rounding; the spatial-gaussian
normalization cancels in ker/norm):
  m_u[p]   = exp(-0.5*sum_c (g[c,p+u]-g[c,p])^2) * exp(-0.5*(uy^2+ux^2)/s^2)
  out[c,p] = sum_u m_u[p]*in[c,p+u] / sum_u m_u[p]        (zero padding)

Symmetry m_{-u}[p] = m_u[p-u] -> only 25 maps computed on an extended grid,
all 49 aligned maps then materialized by DMA remap.

Engine plan (v9): DVE runs 47 of the 49 apply products (fp16 TT at 2
elem/cyc) plus the early c-reduce adds; Pool runs guide subs, late
c-reduce adds, 2 late products (dedicated buffers) and the norm tree;
ACT runs squares+exps only; PE accumulates all products into PSUM via
identity matmuls; both input alignment parities are prepared host-side
and loaded with one DMA per (dr,half).
"""

import numpy as np

B, C, H, W = 2, 32, 256, 256
CG = 3
R = 3                      # K//2
NB = 4                     # h-blocks per batch
RB = H // NB               # 64 out rows per core
NCORES = 8

GR = RB + 2 * R            # 70 rows   (out rows -3..66)
GX = W + 4 * R             # 268 guide cols (-6..261)
MR = RB + R                # 67 map rows (-3..63)
MJ = W + 2 * R             # 262 map cols (-3..258)
MS = W + 2 * R             # 262 per-map stride in K25 (even)
WH = 2                     # w halves in apply layout
XC = W // WH               # 128
XW = XC + 2 * R            # 134 per-half x window

UPLUS = [(0, ux) for ux in range(0, R + 1)] + [
    (uy, ux) for uy in range(1, R + 1) for ux in range(-R, R + 1)
]
ALL_U = []
for uy in range(-R, R + 1):
    for ux in range(-R, R + 1):
        if (uy, ux) in UPLUS:
            ALL_U.append((uy, ux, UPLUS.index((uy, ux)), False))
        else:
            ALL_U.append((uy, ux, UPLUS.index((-uy, -ux)), True))

N_POOL_PRODUCTS = 3        # late products on Pool (dedicated bufs, no reuse)

_COMPILED = None


def _build_nc(legalize=True):
    import concourse.bass as bass
    import concourse.mybir as mybir
    from concourse.bass import AP
    from concourse.tile import TileContext, add_dep_helper

    fp32 = mybir.dt.float32
    fp16 = mybir.dt.float16
    ALU = mybir.AluOpType
    ACTF = mybir.ActivationFunctionType

    nc = bass.Bass()

    guide_d = nc.declare_dram_parameter("guide16", [CG, GR, GX], fp16, isOutput=False)
    inp2_d = nc.declare_dram_parameter("inp2", [2 * 128, 7 * C * XW], fp16,
                                       isOutput=False)
    browt_d = nc.declare_dram_parameter("browt", [128, 25], fp32, isOutput=False)
    ident_d = nc.declare_dram_parameter("ident", [128, 128], fp16, isOutput=False)
    out_d = nc.declare_dram_parameter("out", [C, RB, W], fp32, isOutput=True)

    def sb(t, p0, pn, off, dims):
        """AP into sbuf/psum tensor: partitions [p0, p0+pn), free offset+dims."""
        sl = t[p0:p0 + pn]
        return AP(sl.tensor, sl.offset + off, [sl.ap[0], *dims])

    def dr_ap(d, off, dims):
        full = d[:]
        return AP(full.tensor, full.offset + off, dims)

    from contextlib import ExitStack

    NSLOT = 4
    CXW = C * XW

    with TileContext(nc) as tc, ExitStack() as es:
        def mk(name, shape, dt):
            return es.enter_context(nc.sbuf_tensor(name, shape, dt))

        # both alignment parities in one tensor: even at 0, odd at 7*CXW
        INBB = mk("INBB", [128, 14 * CXW], fp16)     # (par,dr,c,x)
        G4 = mk("G4", [128, 4 * CG * GX], fp16)      # (dy,c,x)
        D3 = mk("D3", [128, 4096], fp16)   # padded: doubles as 4th pool product buf
        SQ = mk("SQ", [128, NSLOT * CG * MJ], fp16)
        T2 = mk("T2", [128, NSLOT * MJ], fp16)
        K25 = mk("K25", [128, 25 * MS], fp16)        # (m,j)
        KA = mk("KA", [128, 49 * XC], fp16)          # (u,x)
        NT = G4                                      # norm tree scratch (G4
        # is dead once the last sub has read it; OverlapTracker fences)
        NORM = mk("NORM", [128, XC], fp32)
        RCP = mk("RCP", [128, XC], fp32)
        BROWT = mk("BROWT", [128, 25], fp32)
        IDENT = mk("IDENT", [128, 128], fp16)
        PBUFS = [mk(f"P{i}T", [128, 4096], fp16) for i in range(3)]
        POOLB = [mk("PL0T", [128, 4096], fp16),
                 mk("PL1T", [128, 4096], fp16)]
        POOLB.append(K25)                            # 3rd pool product buf
        OUTC = K25.bitcast(fp32)                     # finish bufs alias K25
        OUTG = G4.bitcast(fp32)                      # ...and G4 (chunk 3);
        # both are dead long before the finish runs (OverlapTracker fences)
        ACC = es.enter_context(nc.psum_tensor("ACC", [128, 4096], fp32))

        v, s, g, t, sync = nc.vector, nc.scalar, nc.gpsimd, nc.tensor, nc.sync

        # ---- guide load first, issued from the ACT queue (idle early, and
        #      its own first consumer is the first square anyway) ----
        for dy in range(4):
            dst = sb(G4, 0, MR, dy * (CG * GX), [[GX, CG], [1, GX]])
            src = dr_ap(guide_d, dy * GX, [[GX, MR], [GR * GX, CG], [1, GX]])
            (sync if dy == 0 else s).dma_start(out=dst, in_=src)

        # ---- input loads: one DMA per (dr, half) covering both parities ----
        def load_in(dr, half):
            p0 = half * 64
            dst = sb(INBB, p0, 64, dr * CXW, [[7 * CXW, 2], [1, CXW]])
            src = dr_ap(inp2_d, p0 * 7 * CXW + dr * CXW,
                        [[7 * CXW, 64], [128 * 7 * CXW, 2], [1, CXW]])
            sync.dma_start(out=dst, in_=src)

        # ---- constants (BROWT precomputed host-side from sigma) ----
        sync.dma_start(out=BROWT[:], in_=browt_d[:])
        load_in(3, 0)
        load_in(3, 1)
        sync.dma_start(out=IDENT[:], in_=ident_d[:])
        # remaining loads emitted at these map iterations (both halves):
        LOAD_AT = {2: [(4, 0)], 4: [(4, 1)], 6: [(2, 0), (2, 1)],
                   10: [(5, 0), (5, 1)], 12: [(1, 0), (1, 1)],
                   16: [(6, 0), (6, 1)], 18: [(0, 0), (0, 1)]}

        # kA slot order: aligned maps at slot=m, shifted at 24+m
        ka_slot = {}
        for (uy, ux, m, shifted) in ALL_U:
            ka_slot[(uy, ux)] = m if not shifted else 24 + m
        # remap batches keyed by the last map index that completes them;
        # first chunk split (0-1 / 2-3) so products start earlier
        AL_CHUNKS = {}
        CH = 4
        _al_list = [(0, 1), (1, 1), (2, 2)] + [(m0c, min(CH, 25 - m0c))
                                       for m0c in range(CH, 25, CH)]
        for (m0c, nmap) in _al_list:
            AL_CHUNKS.setdefault(m0c + nmap - 1, []).append((m0c, nmap))
        _sh_list = [(0, 1, 3), (1, 4, 4), (1, 8, 3), (2, 11, 4),
                    (2, 15, 3), (3, 18, 4), (3, 22, 3)]
        SH_GROUPS = {}
        _vy_base = {0: 1, 1: 4, 2: 11, 3: 18}
        for (vy, mb, nmap) in _sh_list:
            vxmin = ((-3 if vy >= 1 else 1) + (mb - _vy_base[vy]))
            SH_GROUPS.setdefault(mb + nmap - 1, []).append((vy, mb, nmap, vxmin))

        _al_key = {}
        for (m0c, nmap) in _al_list:
            for mm in range(m0c, m0c + nmap):
                _al_key[mm] = m0c + nmap - 1
        _sh_key = {}
        for (vy, mb, nmap) in _sh_list:
            for mm in range(mb, mb + nmap):
                _sh_key[mm] = mb + nmap - 1

        def ready_at(e):
            uy, ux, m, shifted = e
            k = _al_key[m] if not shifted else _sh_key[m]
            if uy == 2 and k == 11:
                k = 14   # defer dr5 so its input load leaves the early DMA
                         # window to the latency-critical dr3/dr4/dr2 loads
            return k

        # products ordered by readiness; N_POOL_PRODUCTS mid-ready ones are
        # deferred to Pool (emitted after the map loop -> they sit in Pool's
        # stream after its guide work, filling its idle tail)
        order = sorted(ALL_U, key=lambda e: (ready_at(e), e[3], e[2], (3 + e[1]) % 2))
        mid = [e for e in order if 15 <= ready_at(e) <= 19]
        pool_ents = mid[:N_POOL_PRODUCTS]
        pool_set = set(id(e) for e in pool_ents)
        prod_by_map = {}
        for e in order:
            if id(e) not in pool_set:
                prod_by_map.setdefault(ready_at(e), []).append(e)

        deferred = []
        pidx = [0]        # DVE products emitted (PBUF ring index)
        nemit = [0]       # all products emitted
        flushed = [-1]
        exp_ins = {}

        def emit_product(ent):
            uy, ux, m, shifted = ent
            ui = ka_slot[(uy, ux)]
            dr = uy + 3
            par = (3 + ux) % 2       # 1 -> odd base copy
            base = par * 7 * CXW + dr * CXW + (3 + ux) + par
            first = nemit[0] == 0
            last = nemit[0] == 48
            if id(ent) in pool_set:
                P = POOLB[nemit[0] - pidx[0]]
                eng = g
            else:
                P = PBUFS[pidx[0] % 3]
                eng = v
                pidx[0] += 1
            if not last:
                in0 = sb(INBB, 0, 128, base, [[XW, C], [1, XC]])
                in1 = sb(KA, 0, 128, ui * XC, [[0, C], [1, XC]])
                eng.tensor_tensor(out=sb(P, 0, 128, 0, [[XC, C], [1, XC]]),
                                  in0=in0, in1=in1, op=ALU.mult)
                for bk in range(8):
                    t.matmul(ACC[:, bk * 512:(bk + 1) * 512], IDENT[:, :],
                             P[:, bk * 512:(bk + 1) * 512],
                             start=first, stop=False)
            else:
                # split the last product into 4 bank-pair chunks (8 channels
                # each) so the finish TTs pipeline with PE's final matmuls
                for ch in range(4):
                    in0 = sb(INBB, 0, 128, base + 8 * ch * XW,
                             [[XW, 8], [1, XC]])
                    in1 = sb(KA, 0, 128, ui * XC, [[0, 8], [1, XC]])
                    eng.tensor_tensor(
                        out=sb(P, 0, 128, 8 * ch * XC, [[XC, 8], [1, XC]]),
                        in0=in0, in1=in1, op=ALU.mult)
                    for bk in (2 * ch, 2 * ch + 1):
                        t.matmul(ACC[:, bk * 512:(bk + 1) * 512], IDENT[:, :],
                                 P[:, bk * 512:(bk + 1) * 512],
                                 start=False, stop=True)
            nemit[0] += 1

        def flush_pair(m0, nb, emit_products=True):
            """c-reduce adds (batched over nb contiguous slots), exps, then
            remap DMAs + dependent products for all newly-unlocked keys.
            Early pairs reduce on DVE (still lightly loaded); later ones on
            Pool so the map pipeline never queues behind DVE products."""
            mlast = m0 + nb - 1
            s0 = m0 % NSLOT
            red = v if mlast <= 1 else g
            red.tensor_tensor(
                out=sb(T2, 0, MR, s0 * MJ, [[MJ, nb], [1, MJ]]),
                in0=sb(SQ, 0, MR, s0 * CG * MJ, [[CG * MJ, nb], [1, MJ]]),
                in1=sb(SQ, 0, MR, s0 * CG * MJ + MJ,
                       [[CG * MJ, nb], [1, MJ]]),
                op=ALU.add)
            red.tensor_tensor(
                out=sb(T2, 0, MR, s0 * MJ, [[MJ, nb], [1, MJ]]),
                in0=sb(T2, 0, MR, s0 * MJ, [[MJ, nb], [1, MJ]]),
                in1=sb(SQ, 0, MR, s0 * CG * MJ + 2 * MJ,
                       [[CG * MJ, nb], [1, MJ]]),
                op=ALU.add)
            for mm in range(m0, mlast + 1):
                exp_ins[mm] = s.activation(
                    out=sb(K25, 0, MR, mm * MS, [[1, MJ]]),
                    in_=sb(T2, 0, MR, (mm % NSLOT) * MJ, [[1, MJ]]),
                    func=ACTF.Exp, scale=-0.5,
                    bias=BROWT[0:MR, mm:mm + 1])
            for mk in range(flushed[0] + 1, mlast + 1):
                for (m0c, nmap) in AL_CHUNKS.get(mk, ()):
                    for wh in range(WH):
                        dst = sb(KA, wh * 64, 64, m0c * XC,
                                 [[XC, nmap], [1, XC]])
                        srcc = sb(K25, 3, 64, m0c * MS + wh * XC + 3,
                                  [[MS, nmap], [1, XC]])
                        sync.dma_start(out=dst, in_=srcc)
                for (vy, mb, nmap, vxmin) in SH_GROUPS.get(mk, ()):
                    for wh in range(WH):
                        dst = sb(KA, wh * 64, 64, (24 + mb) * XC,
                                 [[XC, nmap], [1, XC]])
                        srcc = sb(K25, 3 - vy, 64,
                                  mb * MS + wh * XC + 3 - vxmin,
                                  [[MS - 1, nmap], [1, XC]])
                        sync.dma_start(out=dst, in_=srcc)
            for mk in range(flushed[0] + 1, mlast + 1):
                for ent in prod_by_map.get(mk, ()):
                    if emit_products:
                        emit_product(ent)
                    else:
                        deferred.append(ent)
            flushed[0] = mlast

        # ---- guide phase: per-map pipeline (Pool sub -> ACT sq ->
        #      adds c-reduce (pair-batched, one iter delayed) -> ACT exp w/
        #      per-map spatial bias) ----
        for m, (uy, ux) in enumerate(UPLUS):
            # sub/sq first (the first three subs run on the still-idle DVE,
            # ahead of any products in its queue), then the pair flush (one
            # iteration late so the reducer never waits on this square),
            # then loads so remap DMAs aren't queued behind them
            sl = m % NSLOT
            in0 = sb(G4, 0, MR, uy * CG * GX + 3 + ux, [[GX, CG], [1, MJ]])
            in1 = sb(G4, 0, MR, 3, [[GX, CG], [1, MJ]])
            d3 = sb(D3, 0, MR, sl * CG * MJ, [[MJ, CG], [1, MJ]])
            sub_eng = v if m <= 8 else g
            sub_eng.tensor_tensor(out=d3, in0=in0, in1=in1, op=ALU.subtract)
            _sq = s.activation(out=sb(SQ, 0, MR, sl * CG * MJ, [[1, CG * MJ]]),
                               in_=sb(D3, 0, MR, sl * CG * MJ, [[1, CG * MJ]]),
                               func=ACTF.Square)
            if m - 5 in exp_ins:
                add_dep_helper(_sq.ins, exp_ins[m - 5].ins, sync=False,
                               reason="keep ACT exps ahead of later squares")

            if m >= 2 and m % 2 == 0:
                flush_pair(m - 2, 2)

            for (ldr, lhalf) in LOAD_AT.get(m, ()):
                load_in(ldr, lhalf)
        flush_pair(22, 2)
        flush_pair(24, 1, emit_products=False)

        # ---- norm + recip emitted BEFORE the last products: Pool computes
        #      the norm right after its guide work, DVE reaches the recip
        #      while late products still stream, so the finish is gated only
        #      on the PSUM stop, not on a tail-end recip ----
        # KA has 49 slots: NT[0:24] = KA[0:24]+KA[24:48]; halve; add slot 48.
        g.tensor_tensor(out=sb(NT, 0, 128, 0, [[1, 24 * XC]]),
                        in0=sb(KA, 0, 128, 0, [[1, 24 * XC]]),
                        in1=sb(KA, 0, 128, 24 * XC, [[1, 24 * XC]]),
                        op=ALU.add)
        g.tensor_tensor(out=sb(NT, 0, 128, 0, [[1, 12 * XC]]),
                        in0=sb(NT, 0, 128, 0, [[1, 12 * XC]]),
                        in1=sb(NT, 0, 128, 12 * XC, [[1, 12 * XC]]),
                        op=ALU.add)
        g.tensor_tensor(out=sb(NT, 0, 128, 0, [[1, 6 * XC]]),
                        in0=sb(NT, 0, 128, 0, [[1, 6 * XC]]),
                        in1=sb(NT, 0, 128, 6 * XC, [[1, 6 * XC]]),
                        op=ALU.add)
        g.tensor_tensor(out=sb(NT, 0, 128, 0, [[1, 3 * XC]]),
                        in0=sb(NT, 0, 128, 0, [[1, 3 * XC]]),
                        in1=sb(NT, 0, 128, 3 * XC, [[1, 3 * XC]]),
                        op=ALU.add)
        g.tensor_tensor(out=sb(NT, 0, 128, 0, [[1, XC]]),
                        in0=sb(NT, 0, 128, 0, [[1, XC]]),
                        in1=sb(NT, 0, 128, XC, [[1, XC]]),
                        op=ALU.add)
        g.tensor_tensor(out=sb(NT, 0, 128, XC, [[1, XC]]),
                        in0=sb(NT, 0, 128, 2 * XC, [[1, XC]]),
                        in1=sb(KA, 0, 128, 48 * XC, [[1, XC]]),
                        op=ALU.add)
        g.tensor_tensor(out=NORM[:, :],
                        in0=sb(NT, 0, 128, 0, [[1, XC]]),
                        in1=sb(NT, 0, 128, XC, [[1, XC]]),
                        op=ALU.add)
        v.reciprocal(out=RCP[:, :], in_=NORM[:, :])

        # pool products (their matmuls sit mid-PE-queue), then the final
        # DVE products
        for ent in pool_ents:
            emit_product(ent)
        for ent in deferred:
            emit_product(ent)
        assert nemit[0] == 49

        # ---- finish: out = acc * rcp (bcast over c), 4 chunks of 8 ch,
        #      all on DVE (Pool cannot read PSUM) ----
        for ch in range(4):
            ot, obuf = (OUTC, ch * 8 * XC) if ch < 3 else (OUTG, 0)
            a_sl = ACC[:, ch * 1024:(ch + 1) * 1024]
            a_ap = AP(a_sl.tensor, a_sl.offset, [a_sl.ap[0], [XC, 8], [1, XC]])
            r_ap = sb(RCP, 0, 128, 0, [[0, 8], [1, XC]])
            o_ap = sb(ot, 0, 128, obuf, [[XC, 8], [1, XC]])
            v.tensor_tensor(out=o_ap, in0=a_ap, in1=r_ap, op=ALU.mult)
            for wh in range(WH):
                srcc = sb(ot, wh * 64, 64, obuf, [[XC, 8], [1, XC]])
                dst = dr_ap(out_d, ch * 8 * RB * W + wh * XC,
                            [[W, 64], [RB * W, 8], [1, XC]])
                sync.dma_start(out=dst, in_=srcc)

    if legalize:
        _legalize_waits(nc)
    return nc


def _legalize_waits(nc):
    """walrus codegen allows 1 sem-wait on DMA instructions (2 elsewhere);
    Tile can emit more. Move excess waits onto InstEventSemaphore nops
    inserted just before, on the same engine (sequencer stalls, then issues)."""
    import concourse.mybir as mybir

    ctr = [0]
    for bb in nc.main_func.blocks:
        out = []
        changed = False
        for ins in bb.instructions:
            cap = 1
            si = ins.sync_info
            waits = list(si.on_wait) if si is not None else []
            if len(waits) > cap:
                keep = waits[:cap]
                extra = waits[cap:]
                while extra:
                    chunk, extra = extra[:1], extra[1:]
                    e = mybir.InstEventSemaphore(
                        name=f"wsplit-{ctr[0]}", ins=[], outs=[])
                    ctr[0] += 1
                    e.engine = ins.engine
                    e.sync_info = mybir.SyncInfo(on_wait=chunk, on_update=[])
                    out.append(e)
                ins.sync_info = mybir.SyncInfo(on_wait=keep, on_update=list(si.on_update))
                changed = True
            out.append(ins)
        if changed:
            bb.instructions = out
    return nc


def _host_prep(input, input_for_kernel, sigma_for_kernel):
    inp = np.asarray(input, dtype=np.float32)
    gui = np.asarray(input_for_kernel, dtype=np.float32)
    sig = np.float32(np.asarray(sigma_for_kernel).reshape(()))

    # pad rows/cols by 6 each side (covers all slice windows with zeros)
    gp = np.zeros((B, CG, H + 12, W + 12), dtype=np.float16)
    gp[:, :, 6:6 + H, 6:6 + W] = gui
    ip = np.zeros((B, C, H + 12, W + 12), dtype=np.float16)
    ip[:, :, 6:6 + H, 6:6 + W] = inp

    rr = np.array([float(uy * uy + ux * ux) for (uy, ux) in UPLUS],
                  dtype=np.float32)
    browt = np.tile((-0.5 * rr / (sig * sig))[None, :], (128, 1)).astype(
        np.float32)
    ident = np.eye(128, dtype=np.float16)

    in_maps = []
    for core in range(NCORES):
        b, hb = divmod(core, NB)
        r0 = hb * RB
        # guide rows r0-3..r0+66, cols -6..261 -> gp[rows 6+r0-3 .., cols 0:GX]
        gs = gp[b, :, 3 + r0: 3 + r0 + GR, 0:GX]
        # input windows, both parities: even base x-3, odd base x-4
        # value[par, (wh,row), dr, c, x] = in[c, r0+row+dr-3, wh*128+x-3-par]
        inp2 = np.empty((2, 2, 64, 7, C, XW), dtype=np.float16)
        for par in range(2):
            for wh in range(WH):
                x0 = 6 + wh * XC - 3 - par
                for dr in range(7):
                    rlo = 6 + r0 + dr - 3
                    # [C, 64, XW] -> [64, C, XW]
                    inp2[par, wh, :, dr] = ip[b][:, rlo:rlo + 64,
                                                 x0:x0 + XW].transpose(1, 0, 2)
        in_maps.append({
            "guide16": np.ascontiguousarray(gs),
            "inp2": inp2.reshape(2 * 128, 7 * C * XW),
            "browt": browt,
            "ident": ident,
        })
    return in_maps


def kernel(input, input_for_kernel, sigma_for_kernel):
    global _COMPILED
    from concourse.bass_utils import run_bass_kernel_spmd

    if _COMPILED is None:
        _COMPILED = _build_nc()
    nc = _COMPILED

    in_maps = _host_prep(input, input_for_kernel, sigma_for_kernel)
    res = run_bass_kernel_spmd(nc, in_maps, core_ids=list(range(NCORES)))
    out = np.zeros((B, C, H, W), dtype=np.float32)
    for core in range(NCORES):
        b, hb = divmod(core, NB)
        out[b, :, hb * RB:(hb + 1) * RB, :] = res.results[core]["out"]
    return out
